# revision 2
# baseline (speedup 1.0000x reference)
"""Multi-head attention forward on 8 Trainium2 NeuronCores (Bass/Tile).

Problem: B=2, N=2048, D=1024, H=16 heads of dh=64, fp32.

Sharding: tensor-parallel over heads — core c owns heads {2c, 2c+1} and both
batches for projections + attention; an on-device AllToAll then re-shards by
token so each core computes the output projection (full Wo) for its 512-token
slice with no reduction.

Layouts: all activations travel as [feature, token] ("transposed"), so every
matmul contraction lands on the partition axis:
  qT/kT/vT [128, 4096]  (rows 0-63 head A dims, 64-127 head B dims)
  scoresT[m, n] = kT.T @ qT   (softmax axis m = partitions)
  exp via ScalarE (no max subtraction: scores ~ N(0,1), exp is safe in fp32)
  attn@v: lhsT = v_aug [m, 65] (v transposed back per 128-chunk via PE
  transpose, with a ones column appended) so PSUM row 64 accumulates the
  softmax denominators for free.
  normalization: reciprocal of denom row, broadcast across partitions with a
  one-hot selector matmul, applied on VectorE.

Matmuls run in float32r (TF32-like, ~1.5e-4 rel err, full PE rate at free
dim >= 256). fp32 inputs are DMA'd directly into float32r tiles (legal when
the DRAM tensor is declared float32r).
"""
from contextlib import ExitStack

import numpy as np

import concourse.bass as bass
import concourse.tile as tile
from concourse import bacc, mybir
from concourse.bass_utils import run_bass_kernel_spmd
from concourse.masks import make_identity

F32 = mybir.dt.float32
F32R = mybir.dt.float32r

B, N, D, H, DH = 2, 2048, 1024, 16, 64
W = 8                    # cores
TOK = B * N              # 4096 flattened tokens
TPC = TOK // W           # 512 tokens per core after re-shard
HPC = H // W             # 2 heads per core

_CACHE = {}


def build_bass():
    nc = bacc.Bacc("TRN2", target_bir_lowering=False)

    xT_d = nc.declare_dram_parameter("xT", [D, TOK], F32R, isOutput=False)
    wq_d = nc.declare_dram_parameter("wq", [D, 128], F32R, isOutput=False)
    wk_d = nc.declare_dram_parameter("wk", [D, 128], F32R, isOutput=False)
    wv_d = nc.declare_dram_parameter("wv", [D, 128], F32R, isOutput=False)
    wo_d = nc.declare_dram_parameter("wo", [D, D], F32R, isOutput=False)
    bqkv_d = nc.declare_dram_parameter("bqkv", [128, 3], F32, isOutput=False)
    out_d = nc.declare_dram_parameter("out", [TPC, D], F32, isOutput=True)

    a2a_in = nc.dram_tensor("a2a_in", [W, 128, TPC], F32R)
    a2a_out = nc.dram_tensor("a2a_out", [W, 128, TPC], F32R)

    KC = D // 128        # contraction chunks for projections
    TC = TOK // 512      # 512-token chunks (8)
    MCB = N // 128       # m-chunks per batch (16)

    with tile.TileContext(nc) as tc, ExitStack() as ctx:
        sb1 = ctx.enter_context(tc.tile_pool(name="sb1", bufs=1))
        sbx = ctx.enter_context(tc.tile_pool(name="sbx", bufs=2))
        sbe = ctx.enter_context(tc.tile_pool(name="sbe", bufs=2))
        stage1 = ExitStack()
        ps_pj = stage1.enter_context(tc.tile_pool(name="ps_pj", bufs=2, space="PSUM"))

        # ---------- constants ----------
        ident_f = sb1.tile([128, 128], F32, tag="ident_f")
        make_identity(nc, ident_f[:])
        ident = sb1.tile([128, 128], F32R, tag="ident")
        nc.vector.tensor_copy(ident[:], ident_f[:])

        ones_f = sb1.tile([128, 1], F32, tag="ones_f")
        nc.vector.memset(ones_f[:], 1.0)
        ones_r = sb1.tile([128, 1], F32R, tag="ones_r")
        nc.vector.tensor_copy(ones_r[:], ones_f[:])

        sel = sb1.tile([128, 128], F32, tag="sel")
        nc.vector.memset(sel[:], 0.0)
        nc.vector.memset(sel[0:1, 0:64], 1.0)
        nc.vector.memset(sel[64:65, 64:128], 1.0)

        bias = sb1.tile([128, 3], F32, tag="bias")
        nc.sync.dma_start(bias[:], bqkv_d[:])

        # ---------- weights ----------
        wq = sb1.tile([128, KC, 128], F32R, tag="wq")
        wk = sb1.tile([128, KC, 128], F32R, tag="wk")
        wv = sb1.tile([128, KC, 128], F32R, tag="wv")
        for k in range(KC):
            nc.sync.dma_start(wq[:, k, :], wq_d[bass.ts(k, 128), :])
            nc.sync.dma_start(wk[:, k, :], wk_d[bass.ts(k, 128), :])
            nc.sync.dma_start(wv[:, k, :], wv_d[bass.ts(k, 128), :])
        wo = sb1.tile([128, KC, D], F32R, tag="wo")
        for k in range(KC):
            nc.sync.dma_start(wo[:, k, :], wo_d[bass.ts(k, 128), :])

        # ---------- stage 1: projections (qT, kT resident; v -> v_aug) ----------
        qT = sb1.tile([128, TOK], F32R, tag="qT")
        kT = sb1.tile([128, TOK], F32R, tag="kT")
        v_aug = sb1.tile([128, 2 * MCB, 130], F32R, tag="v_aug")

        for t in range(TC):
            xt = sbx.tile([128, KC, 512], F32R, tag="xt")
            for k in range(KC):
                nc.sync.dma_start(xt[:, k, :], xT_d[bass.ts(k, 128), bass.ts(t, 512)])

            tsl = bass.ts(t, 512)
            for name, w_t, bcol, dst in (
                ("q", wq, bias[:, 0:1], qT[:, tsl]),
                ("k", wk, bias[:, 1:2], kT[:, tsl]),
            ):
                pj = ps_pj.tile([128, 512], F32, tag="pj")
                for k in range(KC):
                    nc.tensor.matmul(pj[:], w_t[:, k, :], xt[:, k, :],
                                     start=(k == 0), stop=(k == KC - 1))
                nc.vector.tensor_scalar_add(dst, pj[:], bcol)

            pj = ps_pj.tile([128, 512], F32, tag="pj")
            for k in range(KC):
                nc.tensor.matmul(pj[:], wv[:, k, :], xt[:, k, :],
                                 start=(k == 0), stop=(k == KC - 1))
            vt = sbx.tile([128, 512], F32R, tag="vt")
            nc.vector.tensor_scalar_add(vt[:], pj[:], bias[:, 2:3])
            # transpose v into v_aug rows (4 m-chunks per 512-token group)
            for i in range(4):
                gm = 4 * t + i
                tp = ps_pj.tile([128, 128], F32R, tag="tp")
                nc.tensor.transpose(tp[:], vt[:, bass.ts(i, 128)], ident[:])
                nc.vector.tensor_copy(v_aug[:, gm, 0:64], tp[:, 0:64])
                nc.vector.tensor_copy(v_aug[:, gm, 65:129], tp[:, 64:128])
                nc.vector.tensor_copy(v_aug[:, gm, 64:65], ones_r[:])
                nc.vector.tensor_copy(v_aug[:, gm, 129:130], ones_r[:])

        stage1.close()
        # ---------- stage 2: attention ----------
        stage2 = ExitStack()
        ps_sc = stage2.enter_context(tc.tile_pool(name="ps_sc", bufs=1, space="PSUM"))
        ps_ha = stage2.enter_context(tc.tile_pool(name="ps_ha", bufs=1, space="PSUM"))
        heads = sb1.tile([128, TOK], F32R, tag="heads")
        rcp = sb1.tile([128, 1024], F32, tag="rcp")
        nc.vector.memset(rcp[:], 0.0)

        for b in range(B):
            for nh in range(2):                   # 1024-token n-window
                tok0 = 2048 * b + 1024 * nh
                ha0 = ps_ha.tile([65, 1024], F32, tag="ha0")
                ha1 = ps_ha.tile([65, 1024], F32, tag="ha1")
                for mc in range(MCB):
                    gm = MCB * b + mc
                    msl = bass.ts(gm, 128)
                    sc0 = ps_sc.tile([128, 1024], F32, tag="sc0")
                    sc1 = ps_sc.tile([128, 1024], F32, tag="sc1")
                    for q4 in range(2):
                        nsl = bass.ds(tok0 + 512 * q4, 512)
                        psl = bass.ts(q4, 512)
                        nc.tensor.matmul(sc0[:, psl], kT[0:64, msl], qT[0:64, nsl],
                                         start=True, stop=True)
                        nc.tensor.matmul(sc1[:, psl], kT[64:128, msl], qT[64:128, nsl],
                                         start=True, stop=True, tile_position=(64, 0))
                    e0 = sbe.tile([128, 1024], F32R, tag="e0")
                    e1 = sbe.tile([128, 1024], F32R, tag="e1")
                    nc.scalar.activation(e0[:], sc0[:], mybir.ActivationFunctionType.Exp)
                    nc.scalar.activation(e1[:], sc1[:], mybir.ActivationFunctionType.Exp)
                    first, last = (mc == 0), (mc == MCB - 1)
                    for q4 in range(2):
                        psl = bass.ts(q4, 512)
                        nc.tensor.matmul(ha0[:, psl], v_aug[:, gm, 0:65], e0[:, psl],
                                         start=first, stop=last)
                        nc.tensor.matmul(ha1[:, psl], v_aug[:, gm, 65:130], e1[:, psl],
                                         start=first, stop=last)

                # normalize: heads[:, tok0:tok0+1024] = ha / denom(head)
                nc.vector.reciprocal(rcp[0:1, :], ha0[64:65, :])
                nc.vector.reciprocal(rcp[64:65, :], ha1[64:65, :])
                for q4 in range(2):
                    psl = bass.ts(q4, 512)
                    bc = ps_sc.tile([128, 512], F32, tag="sc0")
                    nc.tensor.matmul(bc[:], sel[:], rcp[:, psl], start=True, stop=True)
                    bc_s = sbe.tile([128, 512], F32, tag="bc_s")
                    nc.vector.tensor_copy(bc_s[:], bc[:])
                    hsl = bass.ds(tok0 + 512 * q4, 512)
                    nc.vector.tensor_mul(heads[0:64, hsl], ha0[0:64, psl], bc_s[0:64, :])
                    nc.vector.tensor_mul(heads[64:128, hsl], ha1[0:64, psl], bc_s[64:128, :])

        stage2.close()
        # ---------- stage 3: AllToAll + output projection ----------
        ps_op = ctx.enter_context(tc.tile_pool(name="ps_op", bufs=2, space="PSUM"))
        for j in range(W):
            nc.sync.dma_start(a2a_in[j], heads[:, bass.ts(j, TPC)])
        nc.gpsimd.collective_compute(
            "AllToAll",
            mybir.AluOpType.bypass,
            ins=[a2a_in[:]],
            outs=[a2a_out[:]],
            replica_groups=[list(range(W))],
        )
        hT = sb1.tile([128, W, TPC], F32R, tag="hT")
        for j in range(W):
            nc.sync.dma_start(hT[:, j, :], a2a_out[j])

        for tq in range(TPC // 128):
            for dc in range(2):
                op = ps_op.tile([128, 512], F32, tag="op")
                for k in range(KC):
                    nc.tensor.matmul(op[:], hT[:, k, bass.ts(tq, 128)],
                                     wo[:, k, bass.ts(dc, 512)],
                                     start=(k == 0), stop=(k == KC - 1))
                ot = sbe.tile([128, 512], F32, tag="ot")
                nc.vector.tensor_copy(ot[:], op[:])
                nc.sync.dma_start(out_d[bass.ts(tq, 128), bass.ts(dc, 512)], ot[:])

    nc.compile()
    return nc


def _prep_inputs(x, Wq, bq, Wk, bk, Wv, bv, Wo, bo):
    xT = np.ascontiguousarray(x.reshape(TOK, D).T)
    in_maps = []
    for c in range(W):
        sl = slice(128 * c, 128 * (c + 1))
        bqkv = np.stack([bq[sl] / 8.0, bk[sl], bv[sl]], axis=1).astype(np.float32)
        in_maps.append({
            "xT": xT,
            "wq": np.ascontiguousarray(Wq[:, sl]) / 8.0,
            "wk": np.ascontiguousarray(Wk[:, sl]),
            "wv": np.ascontiguousarray(Wv[:, sl]),
            "wo": Wo,
            "bqkv": np.ascontiguousarray(bqkv),
        })
    return in_maps


def run(x, Wq, bq, Wk, bk, Wv, bv, Wo, bo, **run_kwargs):
    if "nc" not in _CACHE:
        _CACHE["nc"] = build_bass()
    nc = _CACHE["nc"]
    in_maps = _prep_inputs(x, Wq, bq, Wk, bk, Wv, bv, Wo, bo)
    res = run_bass_kernel_spmd(nc, in_maps, list(range(W)), **run_kwargs)
    out = np.concatenate([res.results[c]["out"] for c in range(W)], axis=0)
    out = out.reshape(B, N, D) + bo.astype(np.float32)
    return out.astype(np.float32), res


def kernel(x, Wq, bq, Wk, bk, Wv, bv, Wo, bo):
    x, Wq, bq, Wk, bk, Wv, bv, Wo, bo = (
        np.asarray(a, dtype=np.float32)
        for a in (x, Wq, bq, Wk, bk, Wv, bv, Wo, bo)
    )
    out, _ = run(x, Wq, bq, Wk, bk, Wv, bv, Wo, bo)
    return out
